# revision 3
# baseline (speedup 1.0000x reference)
"""Trainium2 Bass kernel for BranchTeacherLayoutLoss (segment_reduce).

Strategy: shard by segment range (B=512 segments -> 64 per core, which are
contiguous runs of members because segment_ids is sorted). Each core gathers
its members' embedding rows from the full table via SWDGE dma_gather
(int16-indexed, so the table is processed in <=32768-row chunks; each call
gathers <=1024 rows -- the SWDGE descriptor-ring capacity -- rotating over 4
SWDGE queues). Per gathered 128-row group it computes inverse row norms on
ACT/DVE, folds them into a one-hot segment-selection matrix, and accumulates
per-segment direction sums with PE matmuls into PSUM. Per-core [64,2] losses
come back; the host sums them. No collectives needed.
"""
import sys
import types
import numpy as np
from contextlib import ExitStack

if '/opt/trn_rl_repo' not in sys.path:
    sys.path.insert(0, '/opt/trn_rl_repo')

import concourse.bass as bass
import concourse.tile as tile
from concourse import bacc, mybir
from concourse.bass_utils import run_bass_kernel_spmd

F32 = mybir.dt.float32
I16 = mybir.dt.int16
Alu = mybir.AluOpType
Act = mybir.ActivationFunctionType

N_CORES = 8
CHUNK = 32768          # int16 index reach per dma_gather call
CALL = 1024            # max indices per dma_gather (SWDGE ring capacity)
N_QUEUES = 4
DVE_EVERY = 3          # route 1-in-3 group sumsq ops to DVE, rest to ACT


def _plan(member_indices, segment_ids, N, B):
    """Host-side index planning. Returns per-core index/segment layouts and
    the static call plan (shared across cores)."""
    spc = B // N_CORES
    nch = (N + CHUNK - 1) // CHUNK
    idx_all = np.asarray(member_indices).astype(np.int64)
    seg_all = np.asarray(segment_ids).astype(np.int64)

    cores = []
    counts_ck = np.zeros((N_CORES, nch), dtype=np.int64)
    for c in range(N_CORES):
        lo = np.searchsorted(seg_all, c * spc, side='left')
        hi = np.searchsorted(seg_all, (c + 1) * spc, side='left')
        idx = idx_all[lo:hi]
        seg = seg_all[lo:hi] - c * spc
        ck = idx // CHUNK
        order = np.argsort(ck, kind='stable')
        idx, seg, ck = idx[order], seg[order], ck[order]
        counts = np.bincount(seg, minlength=spc).astype(np.float32)
        cores.append({'idx': idx, 'seg': seg, 'ck': ck, 'counts': counts})
        counts_ck[c] = np.bincount(ck, minlength=nch)

    # static per-chunk padded sizes and call splits (identical across cores)
    g_k = []
    calls = []  # list of (chunk_idx, call_size)
    for k in range(nch):
        mx = int(counts_ck[:, k].max())
        gk = ((mx + 127) // 128) * 128 if mx > 0 else 0
        g_k.append(gk)
        rem = gk
        while rem > 0:
            g = min(CALL, rem)
            calls.append((k, g))
            rem -= g

    for c in range(N_CORES):
        d = cores[c]
        idx16_cols = []
        seg_cols = []
        for k in range(nch):
            gk = g_k[k]
            if gk == 0:
                continue
            sel = d['ck'] == k
            n = int(sel.sum())
            loc = (d['idx'][sel] - k * CHUNK).astype(np.int16)
            segk = d['seg'][sel].astype(np.float32)
            idx_pad = np.zeros(gk, dtype=np.int16)
            idx_pad[:n] = loc
            seg_pad = np.full(gk, float(spc), dtype=np.float32)
            seg_pad[:n] = segk
            # idx wrap is PER CALL: [i%16, call_off + i//16]
            pos = 0
            while pos < gk:
                g = min(CALL, gk - pos)
                idx16_cols.append(idx_pad[pos:pos + g].reshape(g // 16, 16).T)
                pos += g
            seg_cols.append(seg_pad.reshape(gk // 128, 128).T)
        d['idx16'] = np.tile(np.concatenate(idx16_cols, axis=1), (8, 1))
        d['segf'] = np.concatenate(seg_cols, axis=1)
    return cores, calls, spc, nch


def _build(N, D, B, calls, spc):
    """Build and compile the SPMD Bass program (identical across cores)."""
    n_groups = sum(g for _, g in calls) // 128
    t_idx = sum(g for _, g in calls) // 16

    nc = bacc.Bacc("TRN2", target_bir_lowering=False, debug=False,
                   num_devices=N_CORES, num_swdge_queues=N_QUEUES)
    emb = nc.dram_tensor("emb", [N, D], F32, kind="ExternalInput")
    idx_in = nc.dram_tensor("idx_in", [128, t_idx], I16, kind="ExternalInput")
    seg_in = nc.dram_tensor("seg_in", [128, n_groups], F32, kind="ExternalInput")
    iota_in = nc.dram_tensor("iota_in", [128, spc], F32, kind="ExternalInput")
    tc_in = nc.dram_tensor("tc_in", [spc, D], F32, kind="ExternalInput")
    tcoh_in = nc.dram_tensor("tcoh_in", [spc, 1], F32, kind="ExternalInput")
    rcnt_in = nc.dram_tensor("rcnt_in", [spc, 1], F32, kind="ExternalInput")
    loss_out = nc.dram_tensor("loss_out", [spc, 2], F32, kind="ExternalOutput")

    with tile.TileContext(nc) as tc_ctx, ExitStack() as ctx:
        meta = ctx.enter_context(tc_ctx.tile_pool(name="meta", bufs=1))
        gpool = ctx.enter_context(tc_ctx.tile_pool(name="gather", bufs=6))
        spool = ctx.enter_context(tc_ctx.tile_pool(name="small", bufs=3))
        qpool = ctx.enter_context(tc_ctx.tile_pool(name="scratch", bufs=3))
        ppool = ctx.enter_context(tc_ctx.tile_pool(name="psum", bufs=1, space="PSUM"))
        fpool = ctx.enter_context(tc_ctx.tile_pool(name="final", bufs=1))

        idxt = meta.tile([128, t_idx], I16)
        nc.sync.dma_start(idxt[:], idx_in.ap()[:, :])
        segt = meta.tile([128, n_groups], F32)
        nc.sync.dma_start(segt[:], seg_in.ap()[:, :])
        iot = meta.tile([128, spc], F32)
        nc.sync.dma_start(iot[:], iota_in.ap()[:, :])
        tcv = meta.tile([spc, D], F32)
        nc.sync.dma_start(tcv[:], tc_in.ap()[:, :])
        tco = meta.tile([spc, 1], F32)
        nc.sync.dma_start(tco[:], tcoh_in.ap()[:, :])
        rcn = meta.tile([spc, 1], F32)
        nc.sync.dma_start(rcn[:], rcnt_in.ap()[:, :])

        psum = ppool.tile([spc, D], F32, space="PSUM")
        iota3 = iot[:].rearrange('p (o b) -> p o b', o=1)

        g_all = 0   # global group counter
        coff = 0    # idx tile column offset (int16 cols)
        for ci, (k, gcall) in enumerate(calls):
            r0 = k * CHUNK
            rows = min(CHUNK, N - r0)
            w = gcall // 128  # groups in this call (<= 8)
            gt = gpool.tile([128, w, D], F32, tag="gt")
            nc.gpsimd.dma_gather(
                gt[:], emb.ap()[r0:r0 + rows, :],
                idxt[:, coff:coff + gcall // 16], gcall, gcall, D,
                queue_num=ci % N_QUEUES)
            ss = spool.tile([128, 8], F32, tag="ss")
            for j in range(w):
                rhs = gt[:, j, :]
                if (g_all + j) % DVE_EVERY == DVE_EVERY - 1:
                    sq = qpool.tile([128, D], F32, tag="sq_dve")
                    nc.vector.scalar_tensor_tensor(
                        out=sq[:], in0=rhs, scalar=1.0, in1=rhs,
                        op0=Alu.mult, op1=Alu.mult,
                        accum_out=ss[:, j:j + 1])
                else:
                    sq = qpool.tile([128, D], F32, tag="sq_act")
                    nc.scalar.activation(sq[:], rhs, Act.Square,
                                         accum_out=ss[:, j:j + 1])
            nrm = spool.tile([128, 8], F32, tag="nrm")
            nc.scalar.sqrt(nrm[:, :w], ss[:, :w])
            inv = spool.tile([128, 8], F32, tag="inv")
            nc.vector.reciprocal(inv[:, :w], nrm[:, :w])
            s01 = spool.tile([128, 8, spc], F32, tag="s01")
            seg3 = segt[:, g_all:g_all + w].broadcast_to([128, w, spc])
            nc.vector.tensor_tensor(s01[:, :w, :], seg3,
                                    iota3.broadcast_to([128, w, spc]),
                                    op=Alu.is_equal)
            sw = spool.tile([128, 8, spc], F32, tag="sw")
            nc.vector.tensor_tensor(sw[:, :w, :], s01[:, :w, :],
                                    inv[:, :w].broadcast_to([128, w, spc]),
                                    op=Alu.mult)
            for j in range(w):
                nc.tensor.matmul(psum[:], lhsT=sw[:, j, :], rhs=gt[:, j, :],
                                 start=(g_all + j == 0),
                                 stop=(g_all + j == n_groups - 1))
            g_all += w
            coff += gcall // 16

        # endgame: per-segment losses from psum sums
        sums = fpool.tile([spc, D], F32)
        nc.vector.tensor_copy(sums[:], psum[:])
        mean = fpool.tile([spc, D], F32)
        nc.vector.tensor_scalar(mean[:], sums[:], rcn[:], None, op0=Alu.mult)
        scr = fpool.tile([spc, D], F32)
        msq = fpool.tile([spc, 1], F32)
        nc.vector.scalar_tensor_tensor(out=scr[:], in0=mean[:], scalar=1.0,
                                       in1=mean[:], op0=Alu.mult,
                                       op1=Alu.mult, accum_out=msq[:])
        scr2 = fpool.tile([spc, D], F32)
        tcd = fpool.tile([spc, 1], F32)
        nc.vector.scalar_tensor_tensor(out=scr2[:], in0=mean[:], scalar=1.0,
                                       in1=tcv[:], op0=Alu.mult,
                                       op1=Alu.mult, accum_out=tcd[:])
        nrm2 = fpool.tile([spc, 1], F32)
        nc.scalar.sqrt(nrm2[:], msq[:])
        den = fpool.tile([spc, 1], F32)
        nc.vector.tensor_scalar(den[:], nrm2[:], 1e-12, None, op0=Alu.max)
        invd = fpool.tile([spc, 1], F32)
        nc.vector.reciprocal(invd[:], den[:])
        # closs = 1 - tcd*invd ; coh = 1 - msq*invd ; coloss = relu(coh - tcoh)
        t0 = fpool.tile([spc, 1], F32)
        nc.vector.tensor_tensor(t0[:], tcd[:], invd[:], op=Alu.mult)
        closs = fpool.tile([spc, 1], F32)
        nc.scalar.activation(closs[:], t0[:], Act.Copy, bias=1.0, scale=-1.0)
        t1 = fpool.tile([spc, 1], F32)
        nc.vector.tensor_tensor(t1[:], msq[:], invd[:], op=Alu.mult)
        coh = fpool.tile([spc, 1], F32)
        nc.scalar.activation(coh[:], t1[:], Act.Copy, bias=1.0, scale=-1.0)
        t2 = fpool.tile([spc, 1], F32)
        nc.vector.tensor_tensor(t2[:], coh[:], tco[:], op=Alu.subtract)
        coloss = fpool.tile([spc, 1], F32)
        nc.vector.tensor_scalar(coloss[:], t2[:], 0.0, None, op0=Alu.max)
        out2 = fpool.tile([spc, 2], F32)
        nc.vector.tensor_copy(out2[:, 0:1], closs[:])
        nc.vector.tensor_copy(out2[:, 1:2], coloss[:])
        nc.sync.dma_start(loss_out.ap()[:, :], out2[:])

    nc.compile()
    return nc


def _prepare(embeddings, teacher_centroids, teacher_cohesion,
             member_indices, segment_ids):
    emb = np.ascontiguousarray(np.asarray(embeddings, dtype=np.float32))
    tcv = np.ascontiguousarray(np.asarray(teacher_centroids, dtype=np.float32))
    tcoh = np.asarray(teacher_cohesion, dtype=np.float32)
    N, D = emb.shape
    B = tcv.shape[0]
    cores, calls, spc, nch = _plan(member_indices, segment_ids, N, B)
    nc = _build(N, D, B, calls, spc)
    iota = np.tile(np.arange(spc, dtype=np.float32), (128, 1))
    in_maps = []
    for c in range(N_CORES):
        d = cores[c]
        in_maps.append({
            "emb": emb,
            "idx_in": np.ascontiguousarray(d['idx16']),
            "seg_in": np.ascontiguousarray(d['segf']),
            "iota_in": iota,
            "tc_in": np.ascontiguousarray(tcv[c * spc:(c + 1) * spc]),
            "tcoh_in": np.ascontiguousarray(tcoh[c * spc:(c + 1) * spc, None]),
            "rcnt_in": np.ascontiguousarray(
                (1.0 / np.maximum(d['counts'], 1.0))[:, None]),
        })
    return nc, in_maps, B


def _finish(results, B):
    total = 0.0
    for r in results:
        total += float(r["loss_out"].astype(np.float64).sum())
    return np.array(total / B, dtype=np.float32)


def kernel(embeddings, teacher_centroids, teacher_cohesion,
           member_indices, segment_ids, num_segments=None, **_ignored):
    nc, in_maps, B = _prepare(embeddings, teacher_centroids, teacher_cohesion,
                              member_indices, segment_ids)
    res = run_bass_kernel_spmd(nc, in_maps, core_ids=list(range(N_CORES)))
    return _finish(res.results, B)


def run_traced(embeddings, teacher_centroids, teacher_cohesion,
               member_indices, segment_ids, num_segments=None,
               tmpdir=None, **_ignored):
    """Like kernel() but with NTFF profiling; returns (loss, BassKernelResults)."""
    _install_ntff_hook()
    nc, in_maps, B = _prepare(embeddings, teacher_centroids, teacher_cohesion,
                              member_indices, segment_ids)
    res = run_bass_kernel_spmd(nc, in_maps, core_ids=list(range(N_CORES)),
                               trace=True, tmpdir=tmpdir)
    return _finish(res.results, B), res


def _install_ntff_hook():
    try:
        import antenv
        from trn_agent_boot.trn_boot import _ntff_profile_via_ctypes
    except ImportError:
        return
    if 'antenv.axon_hooks' in sys.modules:
        return
    hook = _ntff_profile_via_ctypes('/opt/axon/libaxon_pjrt.so')
    mod = types.ModuleType('antenv.axon_hooks')
    mod.get_axon_ntff_profile_hook = lambda: hook
    mod.set_axon_ntff_profile_hook = lambda h: None
    sys.modules['antenv.axon_hooks'] = mod
    antenv.axon_hooks = mod


# revision 4
# speedup vs baseline: 1.0350x; 1.0350x over previous
"""Trainium2 Bass kernel for BranchTeacherLayoutLoss (segment_reduce).

Strategy: shard by segment range (B=512 segments -> 64 per core, which are
contiguous runs of members because segment_ids is sorted). Each core gathers
its members' embedding rows from the full table via SWDGE dma_gather
(int16-indexed, so the table is processed in <=32768-row chunks; each call
gathers <=1024 rows -- the SWDGE descriptor-ring capacity -- rotating over 4
SWDGE queues). Per gathered 128-row group it computes inverse row norms on
ACT/DVE, folds them into a one-hot segment-selection matrix, and accumulates
per-segment direction sums with PE matmuls into PSUM. Per-core [64,2] losses
come back; the host sums them. No collectives needed.
"""
import sys
import types
import numpy as np
from contextlib import ExitStack

if '/opt/trn_rl_repo' not in sys.path:
    sys.path.insert(0, '/opt/trn_rl_repo')

import concourse.bass as bass
import concourse.tile as tile
from concourse import bacc, mybir
from concourse.bass_utils import run_bass_kernel_spmd

F32 = mybir.dt.float32
I16 = mybir.dt.int16
Alu = mybir.AluOpType
Act = mybir.ActivationFunctionType

N_CORES = 8
CHUNK = 32768          # int16 index reach per dma_gather call
CALL = 1024            # max indices per dma_gather (SWDGE ring capacity)
N_QUEUES = 4
ACT_PER_CALL = 3       # groups per call whose sumsq runs on ACT (rest on DVE)
BF16 = True            # gather/matmul data path in bf16 (psum accum stays f32)
BF = mybir.dt.bfloat16


def _plan(member_indices, segment_ids, N, B):
    """Host-side index planning. Returns per-core index/segment layouts and
    the static call plan (shared across cores)."""
    spc = B // N_CORES
    nch = (N + CHUNK - 1) // CHUNK
    idx_all = np.asarray(member_indices).astype(np.int64)
    seg_all = np.asarray(segment_ids).astype(np.int64)

    cores = []
    counts_ck = np.zeros((N_CORES, nch), dtype=np.int64)
    for c in range(N_CORES):
        lo = np.searchsorted(seg_all, c * spc, side='left')
        hi = np.searchsorted(seg_all, (c + 1) * spc, side='left')
        idx = idx_all[lo:hi]
        seg = seg_all[lo:hi] - c * spc
        ck = idx // CHUNK
        order = np.argsort(ck, kind='stable')
        idx, seg, ck = idx[order], seg[order], ck[order]
        counts = np.bincount(seg, minlength=spc).astype(np.float32)
        cores.append({'idx': idx, 'seg': seg, 'ck': ck, 'counts': counts})
        counts_ck[c] = np.bincount(ck, minlength=nch)

    # static per-chunk padded sizes and call splits (identical across cores)
    g_k = []
    calls = []  # list of (chunk_idx, call_size)
    for k in range(nch):
        mx = int(counts_ck[:, k].max())
        gk = ((mx + 127) // 128) * 128 if mx > 0 else 0
        g_k.append(gk)
        rem = gk
        while rem > 0:
            g = min(CALL, rem)
            calls.append((k, g))
            rem -= g

    for c in range(N_CORES):
        d = cores[c]
        idx16_cols = []
        seg_cols = []
        for k in range(nch):
            gk = g_k[k]
            if gk == 0:
                continue
            sel = d['ck'] == k
            n = int(sel.sum())
            loc = (d['idx'][sel] - k * CHUNK).astype(np.int16)
            segk = d['seg'][sel].astype(np.float32)
            idx_pad = np.zeros(gk, dtype=np.int16)
            idx_pad[:n] = loc
            seg_pad = np.full(gk, float(spc), dtype=np.float32)
            seg_pad[:n] = segk
            # idx wrap is PER CALL: [i%16, call_off + i//16]
            pos = 0
            while pos < gk:
                g = min(CALL, gk - pos)
                idx16_cols.append(idx_pad[pos:pos + g].reshape(g // 16, 16).T)
                pos += g
            seg_cols.append(seg_pad.reshape(gk // 128, 128).T)
        d['idx16'] = np.tile(np.concatenate(idx16_cols, axis=1), (8, 1))
        d['segf'] = np.concatenate(seg_cols, axis=1)
    return cores, calls, spc, nch


def _build(N, D, B, calls, spc):
    """Build and compile the SPMD Bass program (identical across cores)."""
    n_groups = sum(g for _, g in calls) // 128
    t_idx = sum(g for _, g in calls) // 16

    nc = bacc.Bacc("TRN2", target_bir_lowering=False, debug=False,
                   num_devices=N_CORES, num_swdge_queues=N_QUEUES)
    DT = BF if BF16 else F32
    emb = nc.dram_tensor("emb", [N, D], DT, kind="ExternalInput")
    idx_in = nc.dram_tensor("idx_in", [128, t_idx], I16, kind="ExternalInput")
    seg_in = nc.dram_tensor("seg_in", [128, n_groups], DT, kind="ExternalInput")
    iota_in = nc.dram_tensor("iota_in", [128, spc], DT, kind="ExternalInput")
    tc_in = nc.dram_tensor("tc_in", [spc, D], F32, kind="ExternalInput")
    tcoh_in = nc.dram_tensor("tcoh_in", [spc, 1], F32, kind="ExternalInput")
    rcnt_in = nc.dram_tensor("rcnt_in", [spc, 1], F32, kind="ExternalInput")
    loss_out = nc.dram_tensor("loss_out", [spc, 2], F32, kind="ExternalOutput")

    with tile.TileContext(nc) as tc_ctx, ExitStack() as ctx:
        meta = ctx.enter_context(tc_ctx.tile_pool(name="meta", bufs=1))
        gpool = ctx.enter_context(tc_ctx.tile_pool(name="gather", bufs=6))
        spool = ctx.enter_context(tc_ctx.tile_pool(name="small", bufs=3))
        qpool = ctx.enter_context(tc_ctx.tile_pool(name="scratch", bufs=3))
        ppool = ctx.enter_context(tc_ctx.tile_pool(name="psum", bufs=1, space="PSUM"))
        fpool = ctx.enter_context(tc_ctx.tile_pool(name="final", bufs=1))

        idxt = meta.tile([128, t_idx], I16)
        nc.sync.dma_start(idxt[:], idx_in.ap()[:, :])
        segt = meta.tile([128, n_groups], DT)
        nc.sync.dma_start(segt[:], seg_in.ap()[:, :])
        iot = meta.tile([128, spc], DT)
        nc.sync.dma_start(iot[:], iota_in.ap()[:, :])
        tcv = meta.tile([spc, D], F32)
        nc.sync.dma_start(tcv[:], tc_in.ap()[:, :])
        tco = meta.tile([spc, 1], F32)
        nc.sync.dma_start(tco[:], tcoh_in.ap()[:, :])
        rcn = meta.tile([spc, 1], F32)
        nc.sync.dma_start(rcn[:], rcnt_in.ap()[:, :])

        psum = ppool.tile([spc, D], F32, space="PSUM")
        iota3 = iot[:].rearrange('p (o b) -> p o b', o=1)

        g_all = 0   # global group counter
        coff = 0    # idx tile column offset (int16 cols)
        for ci, (k, gcall) in enumerate(calls):
            r0 = k * CHUNK
            rows = min(CHUNK, N - r0)
            w = gcall // 128  # groups in this call (<= 8)
            gt = gpool.tile([128, w, D], DT, tag="gt")
            nc.gpsimd.dma_gather(
                gt[:], emb.ap()[r0:r0 + rows, :],
                idxt[:, coff:coff + gcall // 16], gcall, gcall, D,
                queue_num=ci % N_QUEUES)
            ss = spool.tile([128, 8], F32, tag="ss")
            for j in range(w):
                rhs = gt[:, j, :]
                if j < ACT_PER_CALL:
                    sq = qpool.tile([128, D], DT, tag="sq_act")
                    nc.scalar.activation(sq[:], rhs, Act.Square,
                                         accum_out=ss[:, j:j + 1])
                else:
                    sq = qpool.tile([128, D], DT, tag="sq_dve")
                    nc.vector.scalar_tensor_tensor(
                        out=sq[:], in0=rhs, scalar=1.0, in1=rhs,
                        op0=Alu.mult, op1=Alu.mult,
                        accum_out=ss[:, j:j + 1])
            nrm = spool.tile([128, 8], F32, tag="nrm")
            nc.scalar.sqrt(nrm[:, :w], ss[:, :w])
            invf = spool.tile([128, 8], F32, tag="invf")
            nc.vector.reciprocal(invf[:, :w], nrm[:, :w])
            inv = spool.tile([128, 8], DT, tag="inv")
            nc.vector.tensor_copy(inv[:, :w], invf[:, :w])
            s01 = spool.tile([128, 8, spc], DT, tag="s01")
            seg3 = segt[:, g_all:g_all + w].broadcast_to([128, w, spc])
            nc.vector.tensor_tensor(s01[:, :w, :], seg3,
                                    iota3.broadcast_to([128, w, spc]),
                                    op=Alu.is_equal)
            sw = spool.tile([128, 8, spc], DT, tag="sw")
            nc.vector.tensor_tensor(sw[:, :w, :], s01[:, :w, :],
                                    inv[:, :w].broadcast_to([128, w, spc]),
                                    op=Alu.mult)
            for j in range(w):
                nc.tensor.matmul(psum[:], lhsT=sw[:, j, :], rhs=gt[:, j, :],
                                 start=(g_all + j == 0),
                                 stop=(g_all + j == n_groups - 1))
            g_all += w
            coff += gcall // 16

        # endgame: per-segment losses from psum sums
        sums = fpool.tile([spc, D], F32)
        nc.vector.tensor_copy(sums[:], psum[:])
        mean = fpool.tile([spc, D], F32)
        nc.vector.tensor_scalar(mean[:], sums[:], rcn[:], None, op0=Alu.mult)
        scr = fpool.tile([spc, D], F32)
        msq = fpool.tile([spc, 1], F32)
        nc.vector.scalar_tensor_tensor(out=scr[:], in0=mean[:], scalar=1.0,
                                       in1=mean[:], op0=Alu.mult,
                                       op1=Alu.mult, accum_out=msq[:])
        scr2 = fpool.tile([spc, D], F32)
        tcd = fpool.tile([spc, 1], F32)
        nc.vector.scalar_tensor_tensor(out=scr2[:], in0=mean[:], scalar=1.0,
                                       in1=tcv[:], op0=Alu.mult,
                                       op1=Alu.mult, accum_out=tcd[:])
        nrm2 = fpool.tile([spc, 1], F32)
        nc.scalar.sqrt(nrm2[:], msq[:])
        den = fpool.tile([spc, 1], F32)
        nc.vector.tensor_scalar(den[:], nrm2[:], 1e-12, None, op0=Alu.max)
        invd = fpool.tile([spc, 1], F32)
        nc.vector.reciprocal(invd[:], den[:])
        # closs = 1 - tcd*invd ; coh = 1 - msq*invd ; coloss = relu(coh - tcoh)
        t0 = fpool.tile([spc, 1], F32)
        nc.vector.tensor_tensor(t0[:], tcd[:], invd[:], op=Alu.mult)
        closs = fpool.tile([spc, 1], F32)
        nc.scalar.activation(closs[:], t0[:], Act.Copy, bias=1.0, scale=-1.0)
        t1 = fpool.tile([spc, 1], F32)
        nc.vector.tensor_tensor(t1[:], msq[:], invd[:], op=Alu.mult)
        coh = fpool.tile([spc, 1], F32)
        nc.scalar.activation(coh[:], t1[:], Act.Copy, bias=1.0, scale=-1.0)
        t2 = fpool.tile([spc, 1], F32)
        nc.vector.tensor_tensor(t2[:], coh[:], tco[:], op=Alu.subtract)
        coloss = fpool.tile([spc, 1], F32)
        nc.vector.tensor_scalar(coloss[:], t2[:], 0.0, None, op0=Alu.max)
        out2 = fpool.tile([spc, 2], F32)
        nc.vector.tensor_copy(out2[:, 0:1], closs[:])
        nc.vector.tensor_copy(out2[:, 1:2], coloss[:])
        nc.sync.dma_start(loss_out.ap()[:, :], out2[:])

    nc.compile()
    return nc


def _prepare(embeddings, teacher_centroids, teacher_cohesion,
             member_indices, segment_ids):
    import ml_dtypes
    np_dt = ml_dtypes.bfloat16 if BF16 else np.float32
    emb = np.ascontiguousarray(np.asarray(embeddings, dtype=np.float32).astype(np_dt))
    tcv = np.ascontiguousarray(np.asarray(teacher_centroids, dtype=np.float32))
    tcoh = np.asarray(teacher_cohesion, dtype=np.float32)
    N, D = emb.shape
    B = tcv.shape[0]
    cores, calls, spc, nch = _plan(member_indices, segment_ids, N, B)
    nc = _build(N, D, B, calls, spc)
    iota = np.tile(np.arange(spc, dtype=np.float32), (128, 1)).astype(np_dt)
    in_maps = []
    for c in range(N_CORES):
        d = cores[c]
        in_maps.append({
            "emb": emb,
            "idx_in": np.ascontiguousarray(d['idx16']),
            "seg_in": np.ascontiguousarray(d['segf'].astype(np_dt)),
            "iota_in": iota,
            "tc_in": np.ascontiguousarray(tcv[c * spc:(c + 1) * spc]),
            "tcoh_in": np.ascontiguousarray(tcoh[c * spc:(c + 1) * spc, None]),
            "rcnt_in": np.ascontiguousarray(
                (1.0 / np.maximum(d['counts'], 1.0))[:, None]),
        })
    return nc, in_maps, B


def _finish(results, B):
    total = 0.0
    for r in results:
        total += float(r["loss_out"].astype(np.float64).sum())
    return np.array(total / B, dtype=np.float32)


def kernel(embeddings, teacher_centroids, teacher_cohesion,
           member_indices, segment_ids, num_segments=None, **_ignored):
    nc, in_maps, B = _prepare(embeddings, teacher_centroids, teacher_cohesion,
                              member_indices, segment_ids)
    res = run_bass_kernel_spmd(nc, in_maps, core_ids=list(range(N_CORES)))
    return _finish(res.results, B)


def run_traced(embeddings, teacher_centroids, teacher_cohesion,
               member_indices, segment_ids, num_segments=None,
               tmpdir=None, **_ignored):
    """Like kernel() but with NTFF profiling; returns (loss, BassKernelResults)."""
    _install_ntff_hook()
    nc, in_maps, B = _prepare(embeddings, teacher_centroids, teacher_cohesion,
                              member_indices, segment_ids)
    res = run_bass_kernel_spmd(nc, in_maps, core_ids=list(range(N_CORES)),
                               trace=True, tmpdir=tmpdir)
    return _finish(res.results, B), res


def _install_ntff_hook():
    try:
        import antenv
        from trn_agent_boot.trn_boot import _ntff_profile_via_ctypes
    except ImportError:
        return
    if 'antenv.axon_hooks' in sys.modules:
        return
    hook = _ntff_profile_via_ctypes('/opt/axon/libaxon_pjrt.so')
    mod = types.ModuleType('antenv.axon_hooks')
    mod.get_axon_ntff_profile_hook = lambda: hook
    mod.set_axon_ntff_profile_hook = lambda h: None
    sys.modules['antenv.axon_hooks'] = mod
    antenv.axon_hooks = mod


# revision 6
# speedup vs baseline: 1.8373x; 1.7753x over previous
"""Trainium2 Bass kernel for BranchTeacherLayoutLoss (segment_reduce).

Strategy: shard by segment range (B=512 segments -> 64 per core, which are
contiguous runs of members because segment_ids is sorted). Each core gathers
its members' embedding rows from the full table via SWDGE dma_gather
(int16-indexed, so the table is processed in <=32768-row chunks; each call
gathers <=1024 rows -- the SWDGE descriptor-ring capacity -- rotating over 4
SWDGE queues). Per gathered 128-row group it computes inverse row norms on
ACT/DVE, folds them into a one-hot segment-selection matrix, and accumulates
per-segment direction sums with PE matmuls into PSUM. Per-core [64,2] losses
come back; the host sums them. No collectives needed.
"""
import sys
import types
import numpy as np
from contextlib import ExitStack

if '/opt/trn_rl_repo' not in sys.path:
    sys.path.insert(0, '/opt/trn_rl_repo')

import concourse.bass as bass
import concourse.tile as tile
from concourse import bacc, mybir
from concourse.bass_utils import run_bass_kernel_spmd

F32 = mybir.dt.float32
I16 = mybir.dt.int16
Alu = mybir.AluOpType
Act = mybir.ActivationFunctionType

N_CORES = 8
CHUNK = 32768          # int16 index reach per dma_gather call
CALL = 1024            # max indices per dma_gather (SWDGE ring capacity)
N_QUEUES = 4
ACT_PER_CALL = 3       # groups per call whose sumsq runs on ACT (rest on DVE)
BF16 = True            # gather/matmul data path in bf16 (psum accum stays f32)
import os as _os
SKIP_COMPUTE = _os.environ.get('SKIP_COMPUTE', '0') == '1'
COMPUTE_MODE = _os.environ.get('COMPUTE_MODE', 'full')  # full|act|dve|mm
BF = mybir.dt.bfloat16


def _plan(member_indices, segment_ids, N, B):
    """Host-side index planning. Returns per-core index/segment layouts and
    the static call plan (shared across cores)."""
    spc = B // N_CORES
    nch = (N + CHUNK - 1) // CHUNK
    idx_all = np.asarray(member_indices).astype(np.int64)
    seg_all = np.asarray(segment_ids).astype(np.int64)

    cores = []
    counts_ck = np.zeros((N_CORES, nch), dtype=np.int64)
    for c in range(N_CORES):
        lo = np.searchsorted(seg_all, c * spc, side='left')
        hi = np.searchsorted(seg_all, (c + 1) * spc, side='left')
        idx = idx_all[lo:hi]
        seg = seg_all[lo:hi] - c * spc
        ck = idx // CHUNK
        order = np.argsort(ck, kind='stable')
        idx, seg, ck = idx[order], seg[order], ck[order]
        counts = np.bincount(seg, minlength=spc).astype(np.float32)
        cores.append({'idx': idx, 'seg': seg, 'ck': ck, 'counts': counts})
        counts_ck[c] = np.bincount(ck, minlength=nch)

    # static per-chunk padded sizes and call splits (identical across cores)
    g_k = []
    calls = []  # list of (chunk_idx, call_size)
    for k in range(nch):
        mx = int(counts_ck[:, k].max())
        gk = ((mx + 127) // 128) * 128 if mx > 0 else 0
        g_k.append(gk)
        rem = gk
        while rem > 0:
            g = min(CALL, rem)
            calls.append((k, g))
            rem -= g

    for c in range(N_CORES):
        d = cores[c]
        idx16_cols = []
        seg_cols = []
        for k in range(nch):
            gk = g_k[k]
            if gk == 0:
                continue
            sel = d['ck'] == k
            n = int(sel.sum())
            loc = (d['idx'][sel] - k * CHUNK).astype(np.int16)
            segk = d['seg'][sel].astype(np.float32)
            idx_pad = np.zeros(gk, dtype=np.int16)
            idx_pad[:n] = loc
            seg_pad = np.full(gk, float(spc), dtype=np.float32)
            seg_pad[:n] = segk
            # idx wrap is PER CALL: [i%16, call_off + i//16]
            pos = 0
            while pos < gk:
                g = min(CALL, gk - pos)
                idx16_cols.append(idx_pad[pos:pos + g].reshape(g // 16, 16).T)
                pos += g
            seg_cols.append(seg_pad.reshape(gk // 128, 128).T)
        d['idx16'] = np.tile(np.concatenate(idx16_cols, axis=1), (8, 1))
        d['segf'] = np.concatenate(seg_cols, axis=1)
    return cores, calls, spc, nch


def _build(N, D, B, calls, spc):
    """Build and compile the SPMD Bass program (identical across cores)."""
    n_groups = sum(g for _, g in calls) // 128
    t_idx = sum(g for _, g in calls) // 16

    nc = bacc.Bacc("TRN2", target_bir_lowering=False, debug=False,
                   num_devices=N_CORES, num_swdge_queues=N_QUEUES)
    DT = BF if BF16 else F32
    emb = nc.dram_tensor("emb", [N, D], DT, kind="ExternalInput")
    idx_in = nc.dram_tensor("idx_in", [128, t_idx], I16, kind="ExternalInput")
    seg_in = nc.dram_tensor("seg_in", [128, n_groups], DT, kind="ExternalInput")
    iota_in = nc.dram_tensor("iota_in", [128, spc], DT, kind="ExternalInput")
    tc_in = nc.dram_tensor("tc_in", [spc, D], F32, kind="ExternalInput")
    tcoh_in = nc.dram_tensor("tcoh_in", [spc, 1], F32, kind="ExternalInput")
    rcnt_in = nc.dram_tensor("rcnt_in", [spc, 1], F32, kind="ExternalInput")
    loss_out = nc.dram_tensor("loss_out", [spc, 2], F32, kind="ExternalOutput")

    with tile.TileContext(nc) as tc_ctx, ExitStack() as ctx:
        meta = ctx.enter_context(tc_ctx.tile_pool(name="meta", bufs=1))
        gpool = ctx.enter_context(tc_ctx.tile_pool(name="gather", bufs=6))
        spool = ctx.enter_context(tc_ctx.tile_pool(name="small", bufs=3))
        qpool = ctx.enter_context(tc_ctx.tile_pool(name="scratch", bufs=3))
        ppool = ctx.enter_context(tc_ctx.tile_pool(name="psum", bufs=1, space="PSUM"))
        fpool = ctx.enter_context(tc_ctx.tile_pool(name="final", bufs=1))

        idxt = meta.tile([128, t_idx], I16)
        nc.sync.dma_start(idxt[:], idx_in.ap()[:, :])
        segt = meta.tile([128, n_groups], DT)
        nc.sync.dma_start(segt[:], seg_in.ap()[:, :])
        iot = meta.tile([128, spc], DT)
        nc.sync.dma_start(iot[:], iota_in.ap()[:, :])
        tcv = meta.tile([spc, D], F32)
        nc.sync.dma_start(tcv[:], tc_in.ap()[:, :])
        tco = meta.tile([spc, 1], F32)
        nc.sync.dma_start(tco[:], tcoh_in.ap()[:, :])
        rcn = meta.tile([spc, 1], F32)
        nc.sync.dma_start(rcn[:], rcnt_in.ap()[:, :])

        psum = ppool.tile([spc, D], F32, space="PSUM")
        iota3 = iot[:].rearrange('p (o b) -> p o b', o=1)

        g_all = 0   # global group counter
        coff = 0    # idx tile column offset (int16 cols)
        for ci, (k, gcall) in enumerate(calls):
            r0 = k * CHUNK
            rows = min(CHUNK, N - r0)
            w = gcall // 128  # groups in this call (<= 8)
            gt = gpool.tile([128, w, D], DT, tag="gt")
            nc.gpsimd.dma_gather(
                gt[:], emb.ap()[r0:r0 + rows, :],
                idxt[:, coff:coff + gcall // 16], gcall, gcall, D,
                queue_num=ci % N_QUEUES)
            if SKIP_COMPUTE:
                g_all += w
                coff += gcall // 16
                continue
            if COMPUTE_MODE in ('act', 'dve'):
                ss = spool.tile([128, 8], F32, tag="ss")
                for j in range(w):
                    rhs = gt[:, j, :]
                    if COMPUTE_MODE == 'act':
                        sq = qpool.tile([128, D], DT, tag="sq_act")
                        nc.scalar.activation(sq[:], rhs, Act.Square,
                                             accum_out=ss[:, j:j + 1])
                    else:
                        sq = qpool.tile([128, D], DT, tag="sq_dve")
                        nc.vector.scalar_tensor_tensor(
                            out=sq[:], in0=rhs, scalar=1.0, in1=rhs,
                            op0=Alu.mult, op1=Alu.mult,
                            accum_out=ss[:, j:j + 1])
                g_all += w
                coff += gcall // 16
                continue
            if COMPUTE_MODE == 'mm':
                if g_all == 0:
                    mmw = meta.tile([128, spc], DT)
                    nc.vector.memset(mmw[:], 0.01)
                for j in range(w):
                    nc.tensor.matmul(psum[:], lhsT=mmw[:], rhs=gt[:, j, :],
                                     start=(g_all + j == 0),
                                     stop=(g_all + j == n_groups - 1))
                g_all += w
                coff += gcall // 16
                continue
            ss = spool.tile([128, 8], F32, tag="ss")
            for j in range(w):
                rhs = gt[:, j, :]
                if j < ACT_PER_CALL:
                    sq = qpool.tile([128, D], DT, tag="sq_act")
                    nc.scalar.activation(sq[:], rhs, Act.Square,
                                         accum_out=ss[:, j:j + 1])
                else:
                    sq = qpool.tile([128, D], DT, tag="sq_dve")
                    nc.vector.scalar_tensor_tensor(
                        out=sq[:], in0=rhs, scalar=1.0, in1=rhs,
                        op0=Alu.mult, op1=Alu.mult,
                        accum_out=ss[:, j:j + 1])
            nrm = spool.tile([128, 8], F32, tag="nrm")
            nc.scalar.sqrt(nrm[:, :w], ss[:, :w])
            invf = spool.tile([128, 8], F32, tag="invf")
            nc.vector.reciprocal(invf[:, :w], nrm[:, :w])
            inv = spool.tile([128, 8], DT, tag="inv")
            nc.vector.tensor_copy(inv[:, :w], invf[:, :w])
            s01 = spool.tile([128, 8, spc], DT, tag="s01")
            seg3 = segt[:, g_all:g_all + w].broadcast_to([128, w, spc])
            nc.vector.tensor_tensor(s01[:, :w, :], seg3,
                                    iota3.broadcast_to([128, w, spc]),
                                    op=Alu.is_equal)
            sw = spool.tile([128, 8, spc], DT, tag="sw")
            nc.vector.tensor_tensor(sw[:, :w, :], s01[:, :w, :],
                                    inv[:, :w].broadcast_to([128, w, spc]),
                                    op=Alu.mult)
            for j in range(w):
                nc.tensor.matmul(psum[:], lhsT=sw[:, j, :], rhs=gt[:, j, :],
                                 start=(g_all + j == 0),
                                 stop=(g_all + j == n_groups - 1))
            g_all += w
            coff += gcall // 16

        # endgame: per-segment losses from psum sums
        sums = fpool.tile([spc, D], F32)
        if SKIP_COMPUTE or COMPUTE_MODE in ('act', 'dve'):
            nc.vector.memset(sums[:], 0.0)
        else:
            nc.vector.tensor_copy(sums[:], psum[:])
        mean = fpool.tile([spc, D], F32)
        nc.vector.tensor_scalar(mean[:], sums[:], rcn[:], None, op0=Alu.mult)
        scr = fpool.tile([spc, D], F32)
        msq = fpool.tile([spc, 1], F32)
        nc.vector.scalar_tensor_tensor(out=scr[:], in0=mean[:], scalar=1.0,
                                       in1=mean[:], op0=Alu.mult,
                                       op1=Alu.mult, accum_out=msq[:])
        scr2 = fpool.tile([spc, D], F32)
        tcd = fpool.tile([spc, 1], F32)
        nc.vector.scalar_tensor_tensor(out=scr2[:], in0=mean[:], scalar=1.0,
                                       in1=tcv[:], op0=Alu.mult,
                                       op1=Alu.mult, accum_out=tcd[:])
        nrm2 = fpool.tile([spc, 1], F32)
        nc.scalar.sqrt(nrm2[:], msq[:])
        den = fpool.tile([spc, 1], F32)
        nc.vector.tensor_scalar(den[:], nrm2[:], 1e-12, None, op0=Alu.max)
        invd = fpool.tile([spc, 1], F32)
        nc.vector.reciprocal(invd[:], den[:])
        # closs = 1 - tcd*invd ; coh = 1 - msq*invd ; coloss = relu(coh - tcoh)
        t0 = fpool.tile([spc, 1], F32)
        nc.vector.tensor_tensor(t0[:], tcd[:], invd[:], op=Alu.mult)
        closs = fpool.tile([spc, 1], F32)
        nc.scalar.activation(closs[:], t0[:], Act.Copy, bias=1.0, scale=-1.0)
        t1 = fpool.tile([spc, 1], F32)
        nc.vector.tensor_tensor(t1[:], msq[:], invd[:], op=Alu.mult)
        coh = fpool.tile([spc, 1], F32)
        nc.scalar.activation(coh[:], t1[:], Act.Copy, bias=1.0, scale=-1.0)
        t2 = fpool.tile([spc, 1], F32)
        nc.vector.tensor_tensor(t2[:], coh[:], tco[:], op=Alu.subtract)
        coloss = fpool.tile([spc, 1], F32)
        nc.vector.tensor_scalar(coloss[:], t2[:], 0.0, None, op0=Alu.max)
        out2 = fpool.tile([spc, 2], F32)
        nc.vector.tensor_copy(out2[:, 0:1], closs[:])
        nc.vector.tensor_copy(out2[:, 1:2], coloss[:])
        nc.sync.dma_start(loss_out.ap()[:, :], out2[:])

    nc.compile()
    return nc


def _prepare(embeddings, teacher_centroids, teacher_cohesion,
             member_indices, segment_ids):
    import ml_dtypes
    np_dt = ml_dtypes.bfloat16 if BF16 else np.float32
    emb = np.ascontiguousarray(np.asarray(embeddings, dtype=np.float32).astype(np_dt))
    tcv = np.ascontiguousarray(np.asarray(teacher_centroids, dtype=np.float32))
    tcoh = np.asarray(teacher_cohesion, dtype=np.float32)
    N, D = emb.shape
    B = tcv.shape[0]
    cores, calls, spc, nch = _plan(member_indices, segment_ids, N, B)
    nc = _build(N, D, B, calls, spc)
    iota = np.tile(np.arange(spc, dtype=np.float32), (128, 1)).astype(np_dt)
    in_maps = []
    for c in range(N_CORES):
        d = cores[c]
        in_maps.append({
            "emb": emb,
            "idx_in": np.ascontiguousarray(d['idx16']),
            "seg_in": np.ascontiguousarray(d['segf'].astype(np_dt)),
            "iota_in": iota,
            "tc_in": np.ascontiguousarray(tcv[c * spc:(c + 1) * spc]),
            "tcoh_in": np.ascontiguousarray(tcoh[c * spc:(c + 1) * spc, None]),
            "rcnt_in": np.ascontiguousarray(
                (1.0 / np.maximum(d['counts'], 1.0))[:, None]),
        })
    return nc, in_maps, B


def _finish(results, B):
    total = 0.0
    for r in results:
        total += float(r["loss_out"].astype(np.float64).sum())
    return np.array(total / B, dtype=np.float32)


def kernel(embeddings, teacher_centroids, teacher_cohesion,
           member_indices, segment_ids, num_segments=None, **_ignored):
    nc, in_maps, B = _prepare(embeddings, teacher_centroids, teacher_cohesion,
                              member_indices, segment_ids)
    res = run_bass_kernel_spmd(nc, in_maps, core_ids=list(range(N_CORES)))
    return _finish(res.results, B)


def run_traced(embeddings, teacher_centroids, teacher_cohesion,
               member_indices, segment_ids, num_segments=None,
               tmpdir=None, **_ignored):
    """Like kernel() but with NTFF profiling; returns (loss, BassKernelResults)."""
    _install_ntff_hook()
    nc, in_maps, B = _prepare(embeddings, teacher_centroids, teacher_cohesion,
                              member_indices, segment_ids)
    res = run_bass_kernel_spmd(nc, in_maps, core_ids=list(range(N_CORES)),
                               trace=True, tmpdir=tmpdir)
    return _finish(res.results, B), res


def _install_ntff_hook():
    try:
        import antenv
        from trn_agent_boot.trn_boot import _ntff_profile_via_ctypes
    except ImportError:
        return
    if 'antenv.axon_hooks' in sys.modules:
        return
    hook = _ntff_profile_via_ctypes('/opt/axon/libaxon_pjrt.so')
    mod = types.ModuleType('antenv.axon_hooks')
    mod.get_axon_ntff_profile_hook = lambda: hook
    mod.set_axon_ntff_profile_hook = lambda h: None
    sys.modules['antenv.axon_hooks'] = mod
    antenv.axon_hooks = mod
